# revision 12
# baseline (speedup 1.0000x reference)
"""Trainium2 Bass kernel for the ConfidenceGatedModel problem.

Two SPMD launches over 8 NeuronCores:

  Launch 1 — batch-parallel encoder + memory read (4 batches/core):
    per 128-token tile: indirect-DMA embedding gather, PE transpose,
    token-major FFN in fp32r (full PE rate), residual + exact-fp32
    layernorm, exact-fp32 write-gate scores; hidden spilled to DRAM.
    per batch: gpsimd kth_largest yields a threshold strictly between
    score ranks 128/129 (exact top-k selection), masked softmax over
    s = q . hidden, ctx accumulated on PE as sum_t p_t * hidden_t.
    Outputs query_h, unnormalized ctx, and sum(p) per batch.
  Host: ctx = ctx_un / expsum (exact), gathers per-core results.

  Launch 2 — vocab-sharded output projections (4000 cols/core):
    logits_shard = query_h @ do_w_shard + ctx @ out_w_shard (fp32).
  Host: concat shards, add biases, halve.
"""

import numpy as np

import concourse.bacc as bacc
import concourse.bass as bass
import concourse.bass_isa as bass_isa
import concourse.mybir as mybir
from concourse import library_config
from concourse.bass_utils import run_bass_kernel_spmd
from concourse.tile import TileContext

F32 = mybir.dt.float32
F32R = mybir.dt.float32r
I32 = mybir.dt.int32
OP = mybir.AluOpType
AF = mybir.ActivationFunctionType

B, T, H, V = 32, 2048, 1024, 32000
NCORES = 8
BPC = B // NCORES          # batches per core
NT = T // 128              # 128-token tiles per batch
KH = H // 128              # k-tiles over H
K2H = 2 * H // 128         # k-tiles over 2H
VSH = V // NCORES          # vocab shard per core
LN_EPS = 1e-5
NEG_BIG = -1e30


def _bcast_row_ap(dram_tensor, width, parts=128):
    """AP replicating a [1, width] DRAM row across `parts` partitions."""
    a = dram_tensor[:, :]
    return bass.AP(tensor=a.tensor, offset=a.offset, ap=[[0, parts], [1, width]])


def build_launch1(bpc=BPC, t=T, trivial_affine=True):
    """trivial_affine: b1=b2=ln_b=q_b=0 and ln_g=1 (verified on host)."""
    nt = t // 128
    ncand = t - 3
    ksel = min(128, ncand)
    nc = bacc.Bacc("TRN2", target_bir_lowering=False)

    idx = nc.dram_tensor("idx", [bpc, 128, nt], I32, kind="ExternalInput")
    embed = nc.dram_tensor("embed", [V, H], F32, kind="ExternalInput")
    w1s = nc.dram_tensor("w1s", [128, KH, 2 * H], F32, kind="ExternalInput")
    w2s = nc.dram_tensor("w2s", [128, K2H, H], F32, kind="ExternalInput")
    qws = nc.dram_tensor("qws", [128, KH, H], F32, kind="ExternalInput")
    wgrow = nc.dram_tensor("wgrow", [1, H], F32, kind="ExternalInput")
    ident = nc.dram_tensor("ident", [128, 128], F32, kind="ExternalInput")
    padadd = nc.dram_tensor("padadd", [128, 1], F32, kind="ExternalInput")
    if not trivial_affine:
        b1row = nc.dram_tensor("b1row", [1, 2 * H], F32, kind="ExternalInput")
        b2row = nc.dram_tensor("b2row", [1, H], F32, kind="ExternalInput")
        gamrow = nc.dram_tensor("gamrow", [1, H], F32, kind="ExternalInput")
        betrow = nc.dram_tensor("betrow", [1, H], F32, kind="ExternalInput")
        qbrow = nc.dram_tensor("qbrow", [1, H], F32, kind="ExternalInput")
        onesrow = nc.dram_tensor("onesrow", [1, 128], F32, kind="ExternalInput")

    query_h = nc.dram_tensor("query_h", [bpc, H], F32, kind="ExternalOutput")
    ctx_un = nc.dram_tensor("ctx_un", [bpc, H], F32, kind="ExternalOutput")
    expsum = nc.dram_tensor("expsum", [bpc, 1], F32, kind="ExternalOutput")

    from contextlib import ExitStack

    with TileContext(nc) as tc:
        with ExitStack() as ctx:
            consts = ctx.enter_context(tc.tile_pool(name="consts", bufs=1))
            wpool = ctx.enter_context(tc.tile_pool(name="wpool", bufs=1))
            streams = ctx.enter_context(tc.tile_pool(name="streams", bufs=2))
            smalls = ctx.enter_context(tc.tile_pool(name="smalls", bufs=2))
            rowp = ctx.enter_context(tc.tile_pool(name="rowp", bufs=1))
            hqp = ctx.enter_context(tc.tile_pool(name="hqp", bufs=2))
            hTp = ctx.enter_context(tc.tile_pool(name="hTp", bufs=1))
            ff1p = ctx.enter_context(tc.tile_pool(name="ff1p", bufs=1))
            ff1Tp = ctx.enter_context(tc.tile_pool(name="ff1Tp", bufs=1))
            hidp = ctx.enter_context(tc.tile_pool(name="hidp", bufs=2))
            hidrp = ctx.enter_context(tc.tile_pool(name="hidrp", bufs=1))
            tmpp = ctx.enter_context(tc.tile_pool(name="tmpp", bufs=1))
            qfullp = ctx.enter_context(tc.tile_pool(name="qfullp", bufs=1))
            pT = ctx.enter_context(tc.tile_pool(name="pT", bufs=1, space="PSUM"))
            pM1 = ctx.enter_context(tc.tile_pool(name="pM1", bufs=1, space="PSUM"))
            pM2 = ctx.enter_context(tc.tile_pool(name="pM2", bufs=1, space="PSUM"))
            ptail = ctx.enter_context(tc.tile_pool(name="ptail", bufs=1, space="PSUM"))
            dhid = ctx.enter_context(tc.tile_pool(name="dhid", bufs=2, space="DRAM"))
            dqh = ctx.enter_context(tc.tile_pool(name="dqh", bufs=2, space="DRAM"))
            nc.gpsimd.load_library(library_config.attn)

            # ---- resident constants ----
            identity = consts.tile([128, 128], F32)
            nc.sync.dma_start(identity, ident[:, :])
            identity_r = consts.tile([128, 128], F32R)
            nc.vector.tensor_copy(identity_r, identity)
            wgb = consts.tile([128, H], F32)
            nc.sync.dma_start(wgb, _bcast_row_ap(wgrow, H))
            epst = consts.tile([128, 1], F32)
            nc.vector.memset(epst, LN_EPS)
            negbig = consts.tile([128, nt], F32)
            nc.vector.memset(negbig, NEG_BIG)
            padt = consts.tile([128, 1], F32)
            nc.sync.dma_start(padt, padadd[:, :])
            if not trivial_affine:
                gamb = consts.tile([128, H], F32)
                nc.sync.dma_start(gamb, _bcast_row_ap(gamrow, H))
                betb = consts.tile([128, H], F32)
                nc.sync.dma_start(betb, _bcast_row_ap(betrow, H))
                ones_sb = consts.tile([1, 128], F32)
                nc.sync.dma_start(ones_sb, onesrow[:, :])
                ones_r = consts.tile([1, 128], F32R)
                nc.vector.tensor_copy(ones_r, ones_sb)
                b1_sb = consts.tile([1, 2 * H], F32)
                nc.sync.dma_start(b1_sb, b1row[:, :])
                b1_r = consts.tile([1, 2 * H], F32R)
                nc.vector.tensor_copy(b1_r, b1_sb)
                b2_sb = consts.tile([1, H], F32)
                nc.sync.dma_start(b2_sb, b2row[:, :])
                b2_r = consts.tile([1, H], F32R)
                nc.vector.tensor_copy(b2_r, b2_sb)
                qb_sb = consts.tile([1, H], F32)
                nc.sync.dma_start(qb_sb, qbrow[:, :])

            # ---- resident weights, rounded to fp32r ----
            w1r = wpool.tile([128, KH, 2 * H], F32R)
            for j in range(KH):
                for g in range(2):
                    st = streams.tile([128, H], F32, tag="stream")
                    nc.sync.dma_start(st, w1s[:, j, H * g : H * (g + 1)])
                    nc.vector.tensor_copy(w1r[:, j, H * g : H * (g + 1)], st)
            w2r = wpool.tile([128, K2H, H], F32R)
            for j in range(K2H):
                st = streams.tile([128, H], F32, tag="stream")
                nc.sync.dma_start(st, w2s[:, j, :])
                nc.vector.tensor_copy(w2r[:, j, :], st)

            BN_FMAX = nc.vector.BN_STATS_FMAX
            BN_SDIM = nc.vector.BN_STATS_DIM
            BN_ADIM = nc.vector.BN_AGGR_DIM
            nsub = max(1, H // BN_FMAX)

            for b in range(bpc):
                idx_b = smalls.tile([128, nt], I32, tag="idxb")
                nc.sync.dma_start(idx_b, idx[b, :, :])
                scores_b = smalls.tile([128, nt], F32, tag="scores")
                hid_d = dhid.tile([128, nt * H], F32, tag="hidden_dram")
                qh_d = dqh.tile([1, H], F32, tag="qh_dram")

                # ======== phase A: encoder over nt tiles ========
                for c in range(nt):
                    hq = hqp.tile([128, H], F32, tag="hq")
                    nc.gpsimd.indirect_dma_start(
                        out=hq[:, :],
                        out_offset=None,
                        in_=embed[:, :],
                        in_offset=bass.IndirectOffsetOnAxis(
                            ap=idx_b[:, c : c + 1], axis=0
                        ),
                    )
                    # hT = transpose(hq): [H-slice k on partitions, tokens]
                    ps_t = pT.tile([128, H], F32, tag="ptrans")
                    for j in range(KH):
                        nc.tensor.transpose(
                            ps_t[:, 128 * j : 128 * (j + 1)],
                            hq[:, 128 * j : 128 * (j + 1)],
                            identity,
                        )
                    hT = hTp.tile([128, H], F32R, tag="hT")
                    nc.vector.tensor_copy(hT, ps_t)

                    # FFN: two 1024-wide groups over the 2H dim
                    ps2 = pM2.tile([128, H], F32, tag="pmm2")
                    if not trivial_affine:
                        for n in range(2):
                            nc.tensor.matmul(
                                ps2[:, 512 * n : 512 * (n + 1)],
                                ones_r[0:1, :],
                                b2_r[0:1, 512 * n : 512 * (n + 1)],
                                start=True, stop=False,
                            )
                    for g in range(2):
                        ps1 = pM1.tile([128, H], F32, tag="pmm1")
                        if not trivial_affine:
                            for n in range(2):
                                nc.tensor.matmul(
                                    ps1[:, 512 * n : 512 * (n + 1)],
                                    ones_r[0:1, :],
                                    b1_r[0:1, H * g + 512 * n : H * g + 512 * (n + 1)],
                                    start=True, stop=False,
                                )
                        for k in range(KH):
                            for n in range(2):
                                nc.tensor.matmul(
                                    ps1[:, 512 * n : 512 * (n + 1)],
                                    hT[:, 128 * k : 128 * (k + 1)],
                                    w1r[:, k, H * g + 512 * n : H * g + 512 * (n + 1)],
                                    start=(k == 0 and trivial_affine),
                                    stop=(k == KH - 1),
                                )
                        ff1 = ff1p.tile([128, H], F32R, tag="ff1")
                        for n in range(2):
                            nc.scalar.activation(
                                ff1[:, 512 * n : 512 * (n + 1)],
                                ps1[:, 512 * n : 512 * (n + 1)],
                                AF.Relu,
                            )
                        ps_t2 = pT.tile([128, H], F32R, tag="ptrans")
                        for j in range(KH):
                            nc.tensor.transpose(
                                ps_t2[:, 128 * j : 128 * (j + 1)],
                                ff1[:, 128 * j : 128 * (j + 1)],
                                identity_r,
                            )
                        ff1T = ff1Tp.tile([128, H], F32R, tag="ff1T")
                        nc.vector.tensor_copy(ff1T, ps_t2)
                        for k in range(KH):
                            for n in range(2):
                                nc.tensor.matmul(
                                    ps2[:, 512 * n : 512 * (n + 1)],
                                    ff1T[:, 128 * k : 128 * (k + 1)],
                                    w2r[:, KH * g + k, 512 * n : 512 * (n + 1)],
                                    start=(g == 0 and k == 0 and trivial_affine),
                                    stop=(g == 1 and k == KH - 1),
                                )

                    # residual + layernorm (exact fp32)
                    hpre = hidp.tile([128, H], F32, tag="hid")
                    nc.vector.tensor_tensor(hpre, ps2, hq, op=OP.add)
                    stats = smalls.tile([128, nsub, BN_SDIM], F32, tag="bnstats")
                    for s in range(nsub):
                        nc.vector.bn_stats(
                            out=stats[:, s, :],
                            in_=hpre[:, s * BN_FMAX : min(H, (s + 1) * BN_FMAX)],
                        )
                    mv = smalls.tile([128, BN_ADIM], F32, tag="bnmv")
                    nc.vector.bn_aggr(out=mv, in_=stats)
                    # rstd = 1/sqrt(var+eps), Newton-refined to fp32 accuracy
                    sq = smalls.tile([128, 1], F32, tag="sqv")
                    nc.scalar.activation(sq, mv[:, 1:2], AF.Sqrt, bias=epst[:, 0:1])
                    r0 = smalls.tile([128, 1], F32, tag="r0")
                    nc.vector.reciprocal(r0, sq)
                    nt2 = smalls.tile([128, 1], F32, tag="nt2")
                    nc.vector.tensor_tensor(nt2, sq, r0, op=OP.mult)
                    nc.vector.tensor_scalar(nt2, nt2, -1.0, 2.0, op0=OP.mult, op1=OP.add)
                    rstd = smalls.tile([128, 1], F32, tag="rstd")
                    nc.vector.tensor_tensor(rstd, r0, nt2, op=OP.mult)
                    nc.vector.tensor_scalar(
                        hpre, hpre, mv[:, 0:1], rstd[:, 0:1],
                        op0=OP.subtract, op1=OP.mult,
                    )
                    if not trivial_affine:
                        nc.vector.tensor_tensor(hpre, hpre, gamb, op=OP.mult)
                        nc.vector.tensor_tensor(hpre, hpre, betb, op=OP.add)

                    # write-gate score (exact fp32): sum over H of hid*wg
                    sc_tmp = tmpp.tile([128, H], F32, tag="vtmp")
                    nc.vector.scalar_tensor_tensor(
                        out=sc_tmp, in0=hpre, scalar=0.0, in1=wgb,
                        op0=OP.add, op1=OP.mult,
                        accum_out=scores_b[:, c : c + 1],
                    )

                    nc.sync.dma_start(hid_d[:, H * c : H * (c + 1)], hpre)
                    if c == nt - 1:
                        # query position t = T-2 -> tile nt-1, partition 126
                        nc.sync.dma_start(qh_d[:, :], hpre[126:127, :])

                nc.sync.dma_start(query_h[b : b + 1, :], qh_d[:, :])

                # ======== per-batch tail ========
                # q = query_h @ q_w (+q_b), fp32 matmuls, streamed q_w
                qhT = smalls.tile([128, KH], F32, tag="qhT")
                nc.sync.dma_start(
                    qhT,
                    qh_d[:, :].rearrange("one (k p) -> (one p) k", k=KH, p=128),
                )
                ps_q = ptail.tile([1, H], F32, tag="ptail")
                for k in range(KH):
                    qwc = streams.tile([128, H], F32, tag="stream")
                    nc.sync.dma_start(qwc, qws[:, k, :])
                    for n in range(2):
                        nc.tensor.matmul(
                            ps_q[0:1, 512 * n : 512 * (n + 1)],
                            qhT[:, k : k + 1],
                            qwc[:, 512 * n : 512 * (n + 1)],
                            start=(k == 0),
                            stop=(k == KH - 1),
                        )
                q_sb = rowp.tile([1, H], F32, tag="qsb")
                nc.scalar.copy(q_sb, ps_q)
                if not trivial_affine:
                    nc.vector.tensor_tensor(q_sb, q_sb, qb_sb, op=OP.add)
                q_full = qfullp.tile([128, H], F32, tag="qfull")
                nc.gpsimd.partition_broadcast(q_full[:, :], q_sb[0:1, :])

                # pad slots (tokens >= ncand) -> very negative (host mask col)
                pad_c = ncand // 128
                nc.vector.tensor_tensor(
                    scores_b[:, pad_c : pad_c + 1],
                    scores_b[:, pad_c : pad_c + 1],
                    padt,
                    op=OP.add,
                )

                # s_t = q . hidden_t  (exact fp32, from DRAM hidden)
                s_b = smalls.tile([128, nt], F32, tag="sb")
                for c in range(nt):
                    hld = streams.tile([128, H], F32, tag="stream")
                    nc.sync.dma_start(hld, hid_d[:, H * c : H * (c + 1)])
                    st_tmp = tmpp.tile([128, H], F32, tag="vtmp")
                    nc.vector.scalar_tensor_tensor(
                        out=st_tmp, in0=hld, scalar=0.0, in1=q_full,
                        op0=OP.add, op1=OP.mult,
                        accum_out=s_b[:, c : c + 1],
                    )

                # tau strictly between score ranks ksel and ksel+1
                kth = smalls.tile([128, 2], F32, tag="kth")
                qquant = 1.0 - (ksel - 0.5) / float(ncand - 1)
                nc.gpsimd.kth_largest(
                    kth[:, :], scores_b[:, :],
                    n_per_lane=nt, k=min(ksel + 12, 510), quantile=qquant,
                )
                tau_b = smalls.tile([128, 1], F32, tag="taub")
                nc.gpsimd.partition_broadcast(tau_b[:, :], kth[0:1, 0:1])

                # sm = (score > tau) ? s : NEG_BIG
                mask = smalls.tile([128, nt], mybir.dt.uint32, tag="mask")
                nc.vector.tensor_scalar(
                    mask, scores_b, tau_b[:, 0:1], None, op0=OP.is_gt
                )
                sm = smalls.tile([128, nt], F32, tag="sm")
                nc.vector.select(sm, mask, s_b, negbig)

                # m = max(sm); p = exp(sm - m); tot = sum(p)
                mx = smalls.tile([128, 1], F32, tag="mx")
                nc.vector.tensor_reduce(mx, sm, axis=mybir.AxisListType.X, op=OP.max)
                mxg = smalls.tile([128, 1], F32, tag="mxg")
                nc.gpsimd.partition_all_reduce(
                    mxg[:, :], mx[:, :], channels=128,
                    reduce_op=bass_isa.ReduceOp.max,
                )
                negm = smalls.tile([128, 1], F32, tag="negm")
                nc.vector.tensor_scalar(negm, mxg, -1.0, None, op0=OP.mult)
                p_t = smalls.tile([128, nt], F32, tag="pt")
                nc.scalar.activation(p_t, sm, AF.Exp, bias=negm[:, 0:1])
                p_r = smalls.tile([128, nt], F32R, tag="pr")
                nc.vector.tensor_copy(p_r, p_t)
                psums = smalls.tile([128, 1], F32, tag="psums")
                nc.vector.tensor_reduce(
                    psums, p_r[:, :].bitcast(F32), axis=mybir.AxisListType.X,
                    op=OP.add,
                )
                tot = smalls.tile([128, 1], F32, tag="tot")
                nc.gpsimd.partition_all_reduce(
                    tot[:, :], psums[:, :], channels=128,
                    reduce_op=bass_isa.ReduceOp.add,
                )
                nc.sync.dma_start(expsum[b : b + 1, :], tot[0:1, 0:1])

                # ctx_un = sum_t p_t * hidden_t  (fp32r matmuls)
                ps_c = ptail.tile([1, H], F32, tag="ptail")
                for c in range(nt):
                    hld2 = streams.tile([128, H], F32, tag="stream")
                    nc.sync.dma_start(hld2, hid_d[:, H * c : H * (c + 1)])
                    hidr = hidrp.tile([128, H], F32R, tag="hidr")
                    nc.vector.tensor_copy(hidr, hld2)
                    for n in range(2):
                        nc.tensor.matmul(
                            ps_c[0:1, 512 * n : 512 * (n + 1)],
                            p_r[:, c : c + 1],
                            hidr[:, 512 * n : 512 * (n + 1)],
                            start=(c == 0),
                            stop=(c == nt - 1),
                        )
                ctx_sb = rowp.tile([1, H], F32, tag="ctxsb")
                nc.scalar.copy(ctx_sb, ps_c)
                nc.sync.dma_start(ctx_un[b : b + 1, :], ctx_sb)

    nc.compile()
    return nc


def build_launch2(vsh=VSH):
    nc = bacc.Bacc("TRN2", target_bir_lowering=False)
    qhT = nc.dram_tensor("qhT", [128, KH, B], F32, kind="ExternalInput")
    ctxT = nc.dram_tensor("ctxT", [128, KH, B], F32, kind="ExternalInput")
    dw = nc.dram_tensor("dw", [128, KH, vsh], F32, kind="ExternalInput")
    ow = nc.dram_tensor("ow", [128, KH, vsh], F32, kind="ExternalInput")
    logits = nc.dram_tensor("logits", [B, vsh], F32, kind="ExternalOutput")

    nchunk = (vsh + 499) // 500
    with TileContext(nc) as tc:
        with (
            tc.tile_pool(name="qt", bufs=1) as qt,
            tc.tile_pool(name="wstream", bufs=3) as wstream,
            tc.tile_pool(name="outp", bufs=2) as outp,
            tc.tile_pool(name="pp", bufs=2, space="PSUM") as pp,
        ):
            qh_sb = qt.tile([128, KH, B], F32)
            nc.sync.dma_start(qh_sb, qhT[:, :, :])
            ctx_sb = qt.tile([128, KH, B], F32)
            nc.sync.dma_start(ctx_sb, ctxT[:, :, :])
            for v in range(nchunk):
                v0 = v * 500
                vn = min(500, vsh - v0)
                dwc = wstream.tile([128, KH, 500], F32, tag="dwc")
                nc.sync.dma_start(dwc[:, :, :vn], dw[:, :, v0 : v0 + vn])
                owc = wstream.tile([128, KH, 500], F32, tag="owc")
                nc.sync.dma_start(owc[:, :, :vn], ow[:, :, v0 : v0 + vn])
                ps = pp.tile([B, 500], F32, tag="ps")
                for k in range(KH):
                    nc.tensor.matmul(
                        ps[:, :vn], qh_sb[:, k, :], dwc[:, k, :vn],
                        start=(k == 0), stop=False,
                    )
                for k in range(KH):
                    nc.tensor.matmul(
                        ps[:, :vn], ctx_sb[:, k, :], owc[:, k, :vn],
                        start=False, stop=(k == KH - 1),
                    )
                ot = outp.tile([B, 500], F32, tag="ot")
                nc.scalar.copy(ot[:, :vn], ps[:, :vn])
                nc.sync.dma_start(logits[:, v0 : v0 + vn], ot[:, :vn])
    nc.compile()
    return nc


_CACHE = {}
LAST_RESULTS = []
LAST_TIMES = []


def _get_launch1(trivial_affine):
    key = ("l1", trivial_affine)
    if key not in _CACHE:
        _CACHE[key] = build_launch1(trivial_affine=trivial_affine)
    return _CACHE[key]


def _get_launch2():
    if "l2" not in _CACHE:
        _CACHE["l2"] = build_launch2()
    return _CACHE["l2"]


def kernel(seq, embed, w1, b1, w2, b2, ln_g, ln_b, wg_w, wg_b,
           q_w, q_b, out_w, out_b, do_w, do_b):
    seq = np.asarray(seq)
    embed = np.ascontiguousarray(np.asarray(embed, np.float32))
    w1 = np.asarray(w1, np.float32)
    w2 = np.asarray(w2, np.float32)
    q_w = np.asarray(q_w, np.float32)
    wg_w = np.asarray(wg_w, np.float32)
    out_w = np.asarray(out_w, np.float32)
    do_w = np.asarray(do_w, np.float32)
    b1 = np.asarray(b1, np.float32)
    b2 = np.asarray(b2, np.float32)
    ln_g = np.asarray(ln_g, np.float32)
    ln_b = np.asarray(ln_b, np.float32)
    q_b = np.asarray(q_b, np.float32)
    wg_b = np.asarray(wg_b, np.float32)
    out_b = np.asarray(out_b, np.float32)
    do_b = np.asarray(do_b, np.float32)

    trivial_affine = bool(
        not b1.any() and not b2.any() and not ln_b.any()
        and not q_b.any() and bool(np.all(ln_g == 1.0))
    )

    seq_i = seq.astype(np.int32)
    idx_all = seq_i.reshape(B, NT, 128).transpose(0, 2, 1).copy()  # [B,128,NT]
    w1s = w1.reshape(KH, 128, 2 * H).transpose(1, 0, 2).copy()
    w2s = w2.reshape(K2H, 128, H).transpose(1, 0, 2).copy()
    qws = q_w.reshape(KH, 128, H).transpose(1, 0, 2).copy()
    wgrow = np.ascontiguousarray(wg_w.reshape(1, H))
    ident = np.eye(128, dtype=np.float32)
    padcol = np.zeros((128, 1), np.float32)
    padcol[(T - 3) % 128 :, 0] = -2e30

    nc1 = _get_launch1(trivial_affine)
    in_maps = []
    for c in range(NCORES):
        m = {
            "idx": idx_all[BPC * c : BPC * (c + 1)],
            "embed": embed,
            "w1s": w1s,
            "w2s": w2s,
            "qws": qws,
            "wgrow": wgrow,
            "ident": ident,
            "padadd": padcol,
        }
        if not trivial_affine:
            m["b1row"] = np.ascontiguousarray(b1.reshape(1, 2 * H))
            m["b2row"] = np.ascontiguousarray(b2.reshape(1, H))
            m["gamrow"] = np.ascontiguousarray(ln_g.reshape(1, H))
            m["betrow"] = np.ascontiguousarray(ln_b.reshape(1, H))
            m["qbrow"] = np.ascontiguousarray(q_b.reshape(1, H))
            m["onesrow"] = np.ones((1, 128), np.float32)
        in_maps.append(m)
    import time as _time
    _t = _time.time()
    res1 = run_bass_kernel_spmd(nc1, in_maps, core_ids=list(range(NCORES)))
    LAST_TIMES.clear()
    LAST_TIMES.append(_time.time() - _t)
    LAST_RESULTS.clear()
    LAST_RESULTS.append(res1)

    qh_all = np.concatenate([r["query_h"] for r in res1.results], axis=0)
    ctx_un = np.concatenate([r["ctx_un"] for r in res1.results], axis=0)
    esum = np.concatenate([r["expsum"] for r in res1.results], axis=0)
    ctx_all = (ctx_un / esum).astype(np.float32)

    qhT = qh_all.T.reshape(KH, 128, B).transpose(1, 0, 2).copy()
    ctxT = ctx_all.T.reshape(KH, 128, B).transpose(1, 0, 2).copy()
    dwS = do_w.reshape(KH, 128, V).transpose(1, 0, 2)
    owS = out_w.reshape(KH, 128, V).transpose(1, 0, 2)

    nc2 = _get_launch2()
    in_maps2 = [
        {
            "qhT": qhT,
            "ctxT": ctxT,
            "dw": np.ascontiguousarray(dwS[:, :, VSH * c : VSH * (c + 1)]),
            "ow": np.ascontiguousarray(owS[:, :, VSH * c : VSH * (c + 1)]),
        }
        for c in range(NCORES)
    ]
    _t = _time.time()
    res2 = run_bass_kernel_spmd(nc2, in_maps2, core_ids=list(range(NCORES)))
    LAST_TIMES.append(_time.time() - _t)
    LAST_RESULTS.append(res2)
    logits = np.concatenate([r["logits"] for r in res2.results], axis=1)
    out = (logits + (do_b + out_b)[None, :]) * 0.5
    return out.astype(np.float32)
